# revision 1
# baseline (speedup 1.0000x reference)
"""AugmentPipe Trainium2 kernel: flip + affine grid_sample (bilinear, reflect)
+ brightness/contrast/saturation + cutout, data-parallel over 8 NeuronCores.

Strategy:
- Host precomputes, per sample, the exact per-pixel bilinear tap indices and
  weights (replicating the reference's f32 arithmetic with numpy), then ships
  compact per-core metadata tensors.
- Axis-aligned samples (no rotation): the warp is separable -> two one-hot
  f32 matmuls on the PE (vertical then horizontal), exact.
- Rotated samples: 32x32 output blocks, per-block 60x60 input patches; the
  4-tap gather runs on GPSIMD ap_gather, weights applied on DVE.
- Color ops fused on DVE; cutout via a shipped row/col mask.
"""

import numpy as np

B, C, H, W = 64, 3, 512, 512
NCORES = 8
SPC = B // NCORES          # samples per core
BLK = 32                   # rotated-path output block
PR = PW = 58               # rotated-path patch dims
GRID = H // BLK            # 16 blocks per axis
NBLK = GRID * GRID         # 256 blocks per image
ROUNDS = NBLK // 8         # 32 gather rounds (8 groups each)
PXB = BLK * BLK            # 1024 pixels per block
NIDX = 4 * PXB             # weight/value lanes per group per round
NIDXG = PXB                # gather indices per group (1 quad slot per pixel)

TRANSLATE_STD = np.float32(0.125)
SCALE_STD = np.float32(0.2)

_PROGRAM_CACHE = {}


# ---------------------------------------------------------------- host math
def _host_taps(inputs):
    """Per-sample per-pixel tap indices/weights, replicating reference f32 ops."""
    f = np.float32
    u_angle = inputs['u_angle'].astype(f); u_scale = inputs['u_scale'].astype(f)
    u_trans = inputs['u_trans'].astype(f)
    m_rot = inputs['m_rot']; m_scale = inputs['m_scale']; m_trans = inputs['m_trans']
    m_flip = inputs['m_flip']

    angle = np.where(m_rot > 0, (u_angle * f(2.0) - f(1.0)) * f(np.pi), f(0.0)).astype(f)
    sc = np.where(m_scale > 0, (u_scale * f(2.0) - f(1.0)) * SCALE_STD + f(1.0), f(1.0)).astype(f)
    tr = np.where(m_trans > 0, (u_trans * f(2.0) - f(1.0)) * TRANSLATE_STD, f(0.0)).astype(f)
    ca = np.cos(angle).astype(f); sa = np.sin(angle).astype(f)

    lin = np.linspace(f(-1.0), f(1.0), W, dtype=f)
    gx0, gy0 = np.meshgrid(lin, lin, indexing='xy')  # [H, W] f32

    out = []
    for b in range(B):
        gx = (sc[b] * (ca[b] * gx0 - sa[b] * gy0) + tr[b]).astype(f)
        gy = (sc[b] * (sa[b] * gx0 + ca[b] * gy0) + tr[b]).astype(f)
        x = ((gx + f(1.0)) * f(W) - f(1.0)) * f(0.5)
        y = ((gy + f(1.0)) * f(H) - f(1.0)) * f(0.5)

        def reflect(v, size):
            v = np.abs(v + f(0.5))
            v = np.mod(v, f(2.0 * size))
            v = np.minimum(v, f(2.0 * size) - v)
            return np.clip(v - f(0.5), f(0.0), f(size - 1.0)).astype(f)

        x = reflect(x, float(W)); y = reflect(y, float(H))
        x0f = np.floor(x); y0f = np.floor(y)
        wx = (x - x0f).astype(f); wy = (y - y0f).astype(f)
        x0 = np.clip(x0f, 0, W - 1).astype(np.int32)
        x1 = np.clip(x0f + 1, 0, W - 1).astype(np.int32)
        y0 = np.clip(y0f, 0, H - 1).astype(np.int32)
        y1 = np.clip(y0f + 1, 0, H - 1).astype(np.int32)
        if m_flip[b] > 0:  # sample flipped image = mirror tap columns
            x0 = W - 1 - x0
            x1 = W - 1 - x1
        out.append((y0, y1, x0, x1, wy, wx))
    return out


def _axis_matrices(tap):
    """One-hot V/H matrices for an axis-aligned sample. Returns WvT [r,i], Wh [c,j]."""
    y0, y1, x0, x1, wy, wx = tap
    f = np.float32
    Wv = np.zeros((H, H), f)   # [i, r]
    r_i = np.arange(H)
    np.add.at(Wv, (r_i, y0[:, 0]), (f(1.0) - wy[:, 0]))
    np.add.at(Wv, (r_i, y1[:, 0]), wy[:, 0])
    Wh = np.zeros((W, W), f)   # [c, j]
    np.add.at(Wh, (x0[0, :], r_i), (f(1.0) - wx[0, :]))
    np.add.at(Wh, (x1[0, :], r_i), wx[0, :])
    return np.ascontiguousarray(Wv.T), Wh


def _rot_meta(tap, img3, flip):
    """Patches/idx/weights for the rotated path.
    Returns patches [ROUNDS,3,8,PR*PW] f32, idx [ROUNDS,128,NIDX//16] i16,
    w4 [ROUNDS,8,NIDX] f32."""
    y0, y1, x0, x1, wy, wx = tap
    f = np.float32
    xdir = -1 if flip else 1
    patches = np.zeros((ROUNDS, 3, 8, PR * PW, 4), f)
    idxw = np.zeros((ROUNDS, 128, NIDXG // 16), np.int16)
    w4 = np.zeros((ROUNDS, 8, NIDX), f)
    for t in range(ROUNDS):
        for g in range(8):
            b = t * 8 + g
            bi, bj = b // GRID, b % GRID
            sl = (slice(bi * BLK, bi * BLK + BLK), slice(bj * BLK, bj * BLK + BLK))
            by0 = y0[sl].ravel(); by1 = y1[sl].ravel()
            bx0 = x0[sl].ravel(); bx1 = x1[sl].ravel()
            bwy = wy[sl].ravel(); bwx = wx[sl].ravel()
            r0 = int(min(by0.min(), by1.min())); c0 = int(min(bx0.min(), bx1.min()))
            rs = int(max(by0.max(), by1.max())) - r0 + 1
            cs = int(max(bx0.max(), bx1.max())) - c0 + 1
            assert rs <= PR and cs <= PW, (rs, cs)
            crop = np.zeros((3, PR, PW), f)
            cc_ = img3[:, r0:r0 + min(PR, H - r0), c0:c0 + min(PW, W - c0)]
            crop[:, :cc_.shape[1], :cc_.shape[2]] = cc_
            rr = np.arange(PR); jj = np.arange(PW)
            r1c = np.clip(rr + 1, 0, PR - 1); j1c = np.clip(jj + xdir, 0, PW - 1)
            q = patches[t, :, g, :].reshape(3, PR, PW, 4)
            q[:, :, :, 0] = crop
            q[:, :, :, 1] = crop[:, :, j1c]
            q[:, :, :, 2] = crop[:, r1c, :]
            q[:, :, :, 3] = crop[:, r1c][:, :, j1c]
            rel = (by0 - r0) * PW + (bx0 - c0)  # one quad slot per pixel
            idxw[t, 16 * g:16 * g + 16, :] = rel.astype(np.int16).reshape(NIDXG // 16, 16).T
            w4[t, g, :] = np.stack([
                (f(1.0) - bwy) * (f(1.0) - bwx),
                (f(1.0) - bwy) * bwx,
                bwy * (f(1.0) - bwx),
                bwy * bwx,
            ], axis=1).ravel()
    return patches, idxw, w4


def _host_prep(inputs):
    f = np.float32
    taps = _host_taps(inputs)
    m_rot = np.asarray(inputs['m_rot'])
    order = np.argsort(m_rot <= 0, kind='stable')  # rotated samples first
    R = int((m_rot > 0).sum())
    NRS = -(-R // NCORES) if R else 0
    NAS = SPC - NRS

    u_b = inputs['u_bright'].astype(f); u_c = inputs['u_contrast'].astype(f)
    u_s = inputs['u_sat'].astype(f)
    bb = np.where(inputs['m_bright'] > 0, u_b * f(0.2), f(0.0)).astype(f)
    cc = np.where(inputs['m_contrast'] > 0, u_c + f(0.5), f(1.0)).astype(f)
    ss = np.where(inputs['m_sat'] > 0, u_s * f(2.0), f(1.0)).astype(f)
    y0c = np.asarray(inputs['y0']); x0c = np.asarray(inputs['x0'])
    m_cut = np.asarray(inputs['m_cut'])
    images = np.asarray(inputs['images']); noise = np.asarray(inputs['noise'])

    cores = []
    for c in range(NCORES):
        sids = [int(order[k * NCORES + c]) for k in range(SPC)]
        im = np.stack([images[s] for s in sids])
        nz = np.stack([noise[s] for s in sids])
        scal = np.zeros((128, SPC, 8), f)
        cm = np.zeros((SPC, 128, W), f)
        rm = np.zeros((SPC, 128, 4), f)
        for k, s in enumerate(sids):
            m = min(float(cc[s]), 1.0)
            scal[:, k, 0] = cc[s]; scal[:, k, 1] = cc[s] * bb[s]
            scal[:, k, 2] = m; scal[:, k, 3] = ss[s]
            scal[:, k, 4] = (f(1.0) - ss[s]) / f(3.0)
            scal[:, k, 5] = -m
            if m_cut[s] > 0:
                cmv = np.zeros(W, f); cmv[x0c[s]:x0c[s] + W // 2] = 1.0
                rmv = np.zeros(H, f); rmv[y0c[s]:y0c[s] + H // 2] = 1.0
                cm[k] = cmv[None, :]
                rm[k] = rmv.reshape(4, 128).T
        pat = np.zeros((max(NRS, 1), ROUNDS, 3, 8, PR * PW, 4), f)
        idx = np.zeros((max(NRS, 1), ROUNDS, 128, NIDXG // 16), np.int16)
        w4 = np.zeros((max(NRS, 1), ROUNDS, 8, NIDX), f)
        wvT = np.zeros((max(NAS, 1), H, H), f)
        wh = np.zeros((max(NAS, 1), W, W), f)
        for k, s in enumerate(sids):
            if k < NRS:
                pat[k], idx[k], w4[k] = _rot_meta(taps[s], images[s], int(np.asarray(inputs['m_flip'])[s]))
            else:
                wvT[k - NRS], wh[k - NRS] = _axis_matrices(taps[s])
        cores.append(dict(
            imgs=im, noise=nz, scal=scal, cm=cm, rm=rm,
            pat=pat.reshape(pat.shape[0], ROUNDS, 3, 8, -1), idx=idx, w4=w4, wvT=wvT, wh=wh,
            ident=np.eye(128, dtype=f),
        ))
    return cores, [ [int(order[k * NCORES + c]) for k in range(SPC)] for c in range(NCORES)], NRS, NAS


# ---------------------------------------------------------------- device
def _build(NRS, NAS):
    import concourse.bacc as bacc
    import concourse.mybir as mybir
    from concourse import tile

    f32 = mybir.dt.float32
    nc = bacc.Bacc()
    d = {}
    d['imgs'] = nc.dram_tensor('imgs', [SPC, C, H, W], f32, kind='ExternalInput')
    d['noise'] = nc.dram_tensor('noise', [SPC, C, H, W], f32, kind='ExternalInput')
    d['scal'] = nc.dram_tensor('scal', [128, SPC, 8], f32, kind='ExternalInput')
    d['cm'] = nc.dram_tensor('cm', [SPC, 128, W], f32, kind='ExternalInput')
    d['rm'] = nc.dram_tensor('rm', [SPC, 128, 4], f32, kind='ExternalInput')
    d['pat'] = nc.dram_tensor('pat', [max(NRS, 1), ROUNDS, 3, 8, PR * PW * 4], f32, kind='ExternalInput')
    d['idx'] = nc.dram_tensor('idx', [max(NRS, 1), ROUNDS, 128, NIDXG // 16], mybir.dt.int16, kind='ExternalInput')
    d['w4'] = nc.dram_tensor('w4', [max(NRS, 1), ROUNDS, 8, NIDX], f32, kind='ExternalInput')
    d['wvT'] = nc.dram_tensor('wvT', [max(NAS, 1), H, H], f32, kind='ExternalInput')
    d['wh'] = nc.dram_tensor('wh', [max(NAS, 1), W, W], f32, kind='ExternalInput')
    d['ident'] = nc.dram_tensor('ident', [128, 128], f32, kind='ExternalInput')
    out_d = nc.dram_tensor('out', [SPC, C, H, W], f32, kind='ExternalOutput')

    mult = mybir.AluOpType.mult
    add = mybir.AluOpType.add

    with tile.TileContext(nc) as tc:
        with (
            tc.tile_pool(name='dram', bufs=1, space='DRAM') as dpool,
            tc.tile_pool(name='rot', bufs=2) as rpool,
            tc.tile_pool(name='rot1', bufs=1) as r1pool,
            tc.tile_pool(name='ax', bufs=1) as apool,
            tc.tile_pool(name='post', bufs=1) as ppool,
            tc.tile_pool(name='psum', bufs=4, space='PSUM') as pspool,
        ):
            stage = dpool.tile([SPC, C, H, W], f32)
            ident = r1pool.tile([128, 128], f32, tag='ident')
            nc.sync.dma_start(ident[:], d['ident'][:])

            # ---------------- rotated samples ----------------
            for s in range(NRS):
                for t in range(ROUNDS):
                    import dataclasses as _dc
                    P = r1pool.tile([128, PR * PW * 4], f32, tag='P')
                    X4 = rpool.tile([128, NIDX], f32, tag='W4')
                    ix = rpool.tile([128, NIDXG // 16], mybir.dt.int16, tag='ix')
                    for ch in range(C):
                        nc.sync.dma_start(P[ch::16, :], d['pat'][s, t, ch])
                    nc.sync.dma_start(ix[:], d['idx'][s, t])
                    for rep in range(16):
                        nc.scalar.dma_start(X4[rep::16, :], d['w4'][s, t])
                    G = rpool.tile([128, NIDX], f32, tag='G')
                    nc.gpsimd.ap_gather(
                        G[:].rearrange("p (n i) -> p n i", i=4),
                        P[:].rearrange("p (n i) -> p n i", i=4),
                        ix[:], channels=128, num_elems=PR * PW, d=4, num_idxs=NIDXG)
                    nc.vector.tensor_tensor(G[:], G[:], X4[:], op=mult)
                    G4 = G[:].rearrange("p (n k) -> p n k", k=4)
                    S2 = r1pool.tile([128, PXB, 2], f32, tag='S2')
                    nc.vector.tensor_tensor(S2[:], G4[:, :, 0:2], G4[:, :, 2:4], op=add)
                    X = r1pool.tile([128, PXB], f32, tag='X')
                    nc.vector.tensor_tensor(X[:], S2[:, :, 0], S2[:, :, 1], op=add)
                    bi, bj0 = (t * 8) // GRID, (t * 8) % GRID
                    for ch in range(C):
                        dst = stage[s, ch, bi * BLK:(bi + 1) * BLK,
                                    bj0 * BLK:(bj0 + 8) * BLK]
                        nc.sync.dma_start(
                            dst.rearrange("i (g j) -> g i j", g=8),
                            X[ch::16, :].rearrange("g (i j) -> g i j", i=BLK))

            # ---------------- axis-aligned samples ----------------
            for k in range(NAS):
                s = NRS + k
                wv_sb = apool.tile([128, 4, H], f32, tag='wv')
                wh_sb = apool.tile([128, 4, W], f32, tag='wh')
                nc.sync.dma_start(wv_sb[:], d['wvT'][k].rearrange("(t p) i -> p t i", p=128))
                nc.sync.dma_start(wh_sb[:], d['wh'][k].rearrange("(t p) j -> p t j", p=128))
                for ch in range(C):
                    img_sb = apool.tile([128, 4, W], f32, tag='img')
                    nc.sync.dma_start(img_sb[:], d['imgs'][s, ch].rearrange("(t p) c -> p t c", p=128))
                    v_sb = apool.tile([128, 4, W], f32, tag='v')
                    for mi in range(4):
                        vps = pspool.tile([128, W], f32, tag='ps')
                        for kt in range(4):
                            nc.tensor.matmul(
                                vps[:], wv_sb[:, kt, mi * 128:(mi + 1) * 128],
                                img_sb[:, kt, :], start=(kt == 0), stop=(kt == 3))
                        nc.scalar.copy(v_sb[:, mi, :], vps[:])
                    vT_sb = apool.tile([128, 4, H], f32, tag='vt')
                    for ct in range(4):
                        tps = pspool.tile([128, H], f32, tag='ps')
                        for it in range(4):
                            nc.tensor.transpose(
                                tps[:, it * 128:(it + 1) * 128],
                                v_sb[:, it, ct * 128:(ct + 1) * 128], ident[:])
                        nc.scalar.copy(vT_sb[:, ct, :], tps[:])
                    o_sb = apool.tile([128, 4, W], f32, tag='o')
                    for mi in range(4):
                        ops = pspool.tile([128, W], f32, tag='ps')
                        for ct in range(4):
                            nc.tensor.matmul(
                                ops[:], vT_sb[:, ct, mi * 128:(mi + 1) * 128],
                                wh_sb[:, ct, :], start=(ct == 0), stop=(ct == 3))
                        nc.scalar.copy(o_sb[:, mi, :], ops[:])
                    nc.sync.dma_start(
                        stage[s, ch].rearrange("(t p) c -> p t c", p=128), o_sb[:])

            # ---------------- post-ops (uniform) ----------------
            sc_sb = ppool.tile([128, SPC, 8], f32, tag='sc')
            nc.sync.dma_start(sc_sb[:], d['scal'][:])
            for s in range(SPC):
                cm_sb = ppool.tile([128, W], f32, tag='cm')
                rm_sb = ppool.tile([128, 4], f32, tag='rm')
                nc.sync.dma_start(cm_sb[:], d['cm'][s])
                nc.sync.dma_start(rm_sb[:], d['rm'][s])
                Wt = []
                for ch in range(C):
                    w_sb = ppool.tile([128, 4, W], f32, tag=f'w{ch}') if False else apool.tile([128, 4, W], f32, tag=['img','v','vt'][ch])
                    nc.sync.dma_start(w_sb[:], stage[s, ch].rearrange("(t p) c -> p t c", p=128))
                    Wt.append(w_sb)
                nz = []
                for ch in range(C):
                    n_sb = apool.tile([128, 4, W], f32, tag=['o','wv','wh'][ch])
                    nc.sync.dma_start(n_sb[:], d['noise'][s, ch].rearrange("(t p) c -> p t c", p=128))
                    nz.append(n_sb)
                gray = ppool.tile([128, 4, W], f32, tag='gray')
                for ch in range(C):  # brightness+contrast fused + clip
                    nc.vector.tensor_scalar(
                        Wt[ch][:], Wt[ch][:], sc_sb[:, s, 0:1], sc_sb[:, s, 1:2],
                        op0=mult, op1=add)
                    nc.vector.tensor_scalar(
                        Wt[ch][:], Wt[ch][:], sc_sb[:, s, 2:3], sc_sb[:, s, 5:6],
                        op0=mybir.AluOpType.min, op1=mybir.AluOpType.max)
                nc.vector.tensor_tensor(gray[:], Wt[0][:], Wt[1][:], op=add)
                nc.vector.tensor_tensor(gray[:], gray[:], Wt[2][:], op=add)
                nc.vector.tensor_scalar(gray[:], gray[:], sc_sb[:, s, 4:5], None, op0=mult)
                for ch in range(C):  # saturation lerp + clip, then cutout
                    nc.vector.scalar_tensor_tensor(
                        Wt[ch][:], Wt[ch][:], sc_sb[:, s, 3:4], gray[:],
                        op0=mult, op1=add)
                    nc.vector.tensor_scalar(
                        Wt[ch][:], Wt[ch][:], 1.0, -1.0,
                        op0=mybir.AluOpType.min, op1=mybir.AluOpType.max)
                    nc.vector.tensor_tensor(nz[ch][:], nz[ch][:], Wt[ch][:],
                                            op=mybir.AluOpType.subtract)
                    for tt in range(4):
                        nc.vector.tensor_tensor(nz[ch][:, tt, :], nz[ch][:, tt, :],
                                                cm_sb[:], op=mult)
                        nc.vector.scalar_tensor_tensor(
                            Wt[ch][:, tt, :], nz[ch][:, tt, :], rm_sb[:, tt:tt + 1],
                            Wt[ch][:, tt, :], op0=mult, op1=add)
                    nc.sync.dma_start(
                        out_d[s, ch].rearrange("(t p) c -> p t c", p=128), Wt[ch][:])
    nc.compile()
    return nc


def kernel(**inputs):
    from concourse import bass_utils
    cores, sids, NRS, NAS = _host_prep(inputs)
    key = (NRS, NAS)
    if key not in _PROGRAM_CACHE:
        _PROGRAM_CACHE[key] = _build(NRS, NAS)
    nc = _PROGRAM_CACHE[key]
    in_maps = [{k: v for k, v in c.items()} for c in cores]
    res = bass_utils.run_bass_kernel_spmd(nc, in_maps, core_ids=list(range(NCORES)))
    out = np.zeros((B, C, H, W), np.float32)
    for c in range(NCORES):
        o = res.results[c]['out']
        for k, s in enumerate(sids[c]):
            out[s] = o[k]
    return out



# revision 11
# speedup vs baseline: 1.2060x; 1.2060x over previous
"""AugmentPipe Trainium2 kernel: flip + affine grid_sample (bilinear, reflect)
+ brightness/contrast/saturation + cutout, data-parallel over 8 NeuronCores.

v2 strategy:
- Host precomputes per-sample per-pixel bilinear tap indices/weights
  (reference-exact f32 arithmetic in numpy) and ships compact metadata.
- Rotated samples: 32x32 output blocks, 8 blocks (one per Q7 core group)
  per gather round. Per block a padded crop (pitch 58) is DMA'd with 4
  byte-offset variants {0,1,58,59} x 3 channels into 12 of the 16
  partitions of its core group (the spare 4 get a replica so gathered
  values stay finite). One ap_gather (d=1, num_idxs=1024) then fetches
  all 4 bilinear taps for 3 channels of 8 blocks in one instruction --
  the dominant cost, ~2.5 Q7-cycles per index.
- The 4-tap weighted reduction runs on the PE: DVE multiplies the gather
  output by per-tap weights (pre-scaled by the contrast factor on host),
  then a 0/1 stationary matrix reduces 128 partitions to [24 = 3ch x
  8blk, 1024] in PSUM. Brightness/clip/saturation/cutout run fused on
  that tile (DVE, reading PSUM) and the result is DMA'd straight into
  the output stripe. No DRAM staging.
- Axis-aligned samples: separable one-hot V/H matmuls on the PE (exact),
  same fused post-ops, direct output write.
"""

import numpy as np

B, C, H, W = 64, 3, 512, 512
NCORES = 8
SPC = B // NCORES          # samples per core

BLK = 32                   # rot-path output block: 32x32
GRID = H // BLK            # 16x16 blocks
RPS = GRID * GRID // 8     # 32 gather rounds per sample (8 blocks each)
NI = BLK * BLK             # 1024 idx per block
PITCH = 58                 # padded crop pitch (rows and cols)
PLANE = PITCH * PITCH      # 3364 elems per plane
SHIFTS = ((0, 0), (0, 1), (1, 0), (1, 1))  # (dy, dx) per tap plane

TRANSLATE_STD = np.float32(0.125)
SCALE_STD = np.float32(0.2)

_PROGRAM_CACHE = {}


# ---------------------------------------------------------------- host math
def _host_taps(inputs):
    """Per-sample per-pixel tap indices/weights, replicating reference f32 ops."""
    f = np.float32
    u_angle = inputs['u_angle'].astype(f); u_scale = inputs['u_scale'].astype(f)
    u_trans = inputs['u_trans'].astype(f)
    m_rot = inputs['m_rot']; m_scale = inputs['m_scale']; m_trans = inputs['m_trans']
    m_flip = inputs['m_flip']

    angle = np.where(m_rot > 0, (u_angle * f(2.0) - f(1.0)) * f(np.pi), f(0.0)).astype(f)
    sc = np.where(m_scale > 0, (u_scale * f(2.0) - f(1.0)) * SCALE_STD + f(1.0), f(1.0)).astype(f)
    tr = np.where(m_trans > 0, (u_trans * f(2.0) - f(1.0)) * TRANSLATE_STD, f(0.0)).astype(f)
    ca = np.cos(angle).astype(f); sa = np.sin(angle).astype(f)

    lin = np.linspace(f(-1.0), f(1.0), W, dtype=f)
    gx0, gy0 = np.meshgrid(lin, lin, indexing='xy')  # [H, W] f32

    out = []
    for b in range(B):
        gx = (sc[b] * (ca[b] * gx0 - sa[b] * gy0) + tr[b]).astype(f)
        gy = (sc[b] * (sa[b] * gx0 + ca[b] * gy0) + tr[b]).astype(f)
        x = ((gx + f(1.0)) * f(W) - f(1.0)) * f(0.5)
        y = ((gy + f(1.0)) * f(H) - f(1.0)) * f(0.5)

        def reflect(v, size):
            v = np.abs(v + f(0.5))
            v = np.mod(v, f(2.0 * size))
            v = np.minimum(v, f(2.0 * size) - v)
            return np.clip(v - f(0.5), f(0.0), f(size - 1.0)).astype(f)

        x = reflect(x, float(W)); y = reflect(y, float(H))
        x0f = np.floor(x); y0f = np.floor(y)
        wx = (x - x0f).astype(f); wy = (y - y0f).astype(f)
        x0 = np.clip(x0f, 0, W - 1).astype(np.int32)
        x1 = np.clip(x0f + 1, 0, W - 1).astype(np.int32)
        y0 = np.clip(y0f, 0, H - 1).astype(np.int32)
        y1 = np.clip(y0f + 1, 0, H - 1).astype(np.int32)
        if m_flip[b] > 0:  # sample flipped image = mirror tap columns
            x0 = W - 1 - x0
            x1 = W - 1 - x1
        out.append((y0, y1, x0, x1, wy, wx))
    return out


def _axis_matrices(tap):
    """One-hot V/H matrices for an axis-aligned sample. Returns WvT [r,i], Wh [c,j]."""
    y0, y1, x0, x1, wy, wx = tap
    f = np.float32
    Wv = np.zeros((H, H), f)   # [i, r]
    r_i = np.arange(H)
    np.add.at(Wv, (r_i, y0[:, 0]), (f(1.0) - wy[:, 0]))
    np.add.at(Wv, (r_i, y1[:, 0]), wy[:, 0])
    Wh = np.zeros((W, W), f)   # [c, j]
    np.add.at(Wh, (x0[0, :], r_i), (f(1.0) - wx[0, :]))
    np.add.at(Wh, (x1[0, :], r_i), wx[0, :])
    return np.ascontiguousarray(Wv.T), Wh


def _rot_meta(tap, img3, cc_scale):
    """Planes/idx/weights for the rot path of one sample.
    Returns pat [RPS,128,PLANE] f32 (16 shift/channel plane rows per core
    group, dense partitions), idx [RPS,128,NI//16] i16,
    w4 [RPS,128,NI] f32 (pre-scaled by cc_scale, partition-replicated)."""
    y0, y1, x0, x1, wy, wx = tap
    f = np.float32
    pat = np.zeros((RPS, 8, 4, 4, PLANE), f)  # (g, k, c, plane)
    idxw = np.zeros((RPS, 128, NI // 16), np.int16)
    w4 = np.zeros((RPS, 8, 4, NI), f)
    # column-sorted taps: c_lo = min(x0, x1); weights of c_lo / c_lo+1 columns
    c_lo = np.minimum(x0, x1)
    wcl = np.where(x1 == x0, f(1.0),
                   np.where(x1 > x0, f(1.0) - wx, wx)).astype(f)
    wcr = np.where(x1 == x0, f(0.0),
                   np.where(x1 > x0, wx, f(1.0) - wx)).astype(f)
    wrt = np.where(y1 == y0, f(1.0), f(1.0) - wy).astype(f)
    wrb = np.where(y1 == y0, f(0.0), wy).astype(f)

    for t in range(RPS):
        for g in range(8):
            b = t * 8 + g
            bi, bj = b // GRID, b % GRID
            sl = (slice(bi * BLK, (bi + 1) * BLK), slice(bj * BLK, (bj + 1) * BLK))
            by0 = y0[sl].ravel(); by1 = y1[sl].ravel()
            bcl = c_lo[sl].ravel()
            r0 = int(by0.min()); c0 = int(bcl.min())
            rs = int(by1.max()) - r0 + 1
            cs = int(bcl.max()) + 1 - c0 + 1
            assert rs <= PITCH - 1 and cs <= PITCH - 1, (rs, cs)
            crop = np.zeros((3, PITCH + 1, PITCH + 1), f)
            hh = min(PITCH + 1, H - r0); ww = min(PITCH + 1, W - c0)
            crop[:, :hh, :ww] = img3[:, r0:r0 + hh, c0:c0 + ww]
            for k, (dy, dx) in enumerate(SHIFTS):
                for c3 in range(4):
                    pat[t, g, k, c3] = crop[min(c3, 2), dy:dy + PITCH,
                                            dx:dx + PITCH].ravel()
            rel = ((by0 - r0) * PITCH + (bcl - c0)).astype(np.int16)
            idxw[t, 16 * g:16 * g + 16, :] = rel.reshape(NI // 16, 16).T
            w4[t, g, 0, :] = (wrt[sl].ravel() * wcl[sl].ravel()) * cc_scale
            w4[t, g, 1, :] = (wrt[sl].ravel() * wcr[sl].ravel()) * cc_scale
            w4[t, g, 2, :] = (wrb[sl].ravel() * wcl[sl].ravel()) * cc_scale
            w4[t, g, 3, :] = (wrb[sl].ravel() * wcr[sl].ravel()) * cc_scale
    w4rep = np.repeat(w4.reshape(RPS, 32, NI), 4, axis=1).reshape(RPS, 128, NI)
    return pat.reshape(RPS, 128, PLANE), idxw, w4rep


def _host_prep(inputs):
    f = np.float32
    taps = _host_taps(inputs)
    m_rot = np.asarray(inputs['m_rot'])
    order = np.argsort(m_rot <= 0, kind='stable')  # rotated samples first
    R = int((m_rot > 0).sum())
    NRS = -(-R // NCORES) if R else 0
    NAS = SPC - NRS

    u_b = inputs['u_bright'].astype(f); u_c = inputs['u_contrast'].astype(f)
    u_s = inputs['u_sat'].astype(f)
    bb = np.where(inputs['m_bright'] > 0, u_b * f(0.2), f(0.0)).astype(f)
    cc = np.where(inputs['m_contrast'] > 0, u_c + f(0.5), f(1.0)).astype(f)
    ss = np.where(inputs['m_sat'] > 0, u_s * f(2.0), f(1.0)).astype(f)
    y0c = np.asarray(inputs['y0']); x0c = np.asarray(inputs['x0'])
    m_cut = np.asarray(inputs['m_cut'])
    images = np.asarray(inputs['images']); noise = np.asarray(inputs['noise'])

    ys = np.arange(H); xs = np.arange(W)

    # PE reduce matrices: column block c3 maps plane partitions of channel
    # c3 to output partition g (all channels land at partition base 0; the
    # channel axis lives on the PSUM free dim)
    smat = np.zeros((128, 128), f)
    for g in range(8):
        for k in range(4):
            for c3 in range(3):
                smat[16 * g + 4 * k + c3, 8 * c3 + g] = 1.0

    cores = []
    for c in range(NCORES):
        sids = [int(order[k * NCORES + c]) for k in range(SPC)]
        scal = np.zeros((128, SPC, 8), f)
        cm = np.zeros((SPC, 128, W), f)   # axis-path cutout masks
        rm = np.zeros((SPC, 128, 4), f)
        for k, s in enumerate(sids):
            m = min(float(cc[s]), 1.0)
            scal[:, k, 0] = cc[s]; scal[:, k, 1] = cc[s] * bb[s]
            scal[:, k, 2] = m; scal[:, k, 3] = ss[s]
            scal[:, k, 4] = (f(1.0) - ss[s]) / f(3.0)
            scal[:, k, 5] = -m
            if m_cut[s] > 0:
                cmv = np.zeros(W, f); cmv[x0c[s]:x0c[s] + W // 2] = 1.0
                rmv = np.zeros(H, f); rmv[y0c[s]:y0c[s] + H // 2] = 1.0
                cm[k] = cmv[None, :]
                rm[k] = rmv.reshape(4, 128).T

        NR1 = max(NRS, 1); NA1 = max(NAS, 1)
        pat = np.zeros((NR1, RPS, 128, PLANE), f)
        idx = np.zeros((NR1, 128, RPS, NI // 16), np.int16)
        w4 = np.zeros((NR1, RPS, 128, NI), f)
        msk = np.zeros((NR1, RPS, 8, NI), f)
        nzb = np.zeros((NR1, RPS, 8, 3 * NI), f)
        wvT = np.zeros((NA1, H, H), f)
        wh = np.zeros((NA1, W, W), f)
        for k, s in enumerate(sids):
            if k < NRS:
                pat[k], idxw_k, w4[k] = _rot_meta(taps[s], images[s], cc[s])
                idx[k] = idxw_k.transpose(1, 0, 2)
                if m_cut[s] > 0:
                    rmv = (ys >= y0c[s]) & (ys < y0c[s] + H // 2)
                    cmv = (xs >= x0c[s]) & (xs < x0c[s] + W // 2)
                    m2 = (rmv[:, None] & cmv[None, :]).astype(f)  # [H, W]
                    m2 = m2.reshape(GRID, BLK, GRID, BLK).transpose(0, 2, 1, 3)
                    msk[k] = m2.reshape(RPS, 8, NI)
                    nz = noise[s].reshape(3, GRID, BLK, GRID, BLK)
                    nzb[k] = nz.transpose(1, 3, 0, 2, 4).reshape(RPS, 8, 3 * NI)
            else:
                wvT[k - NRS], wh[k - NRS] = _axis_matrices(taps[s])

        imsel = [s for k, s in enumerate(sids) if k >= NRS]
        im = np.stack([images[s] for s in imsel]) if imsel else np.zeros((1, C, H, W), f)
        nzsel = np.stack([noise[s] for s in imsel]) if imsel else np.zeros((1, C, H, W), f)
        cores.append(dict(
            imgs=im, noise=nzsel, scal=scal, cm=cm, rm=rm,
            pat=pat, idx=idx, w4=w4, msk=msk, nzb=nzb, wvT=wvT, wh=wh,
            ident=np.eye(128, dtype=f), smat=smat,
        ))
    return cores, [[int(order[k * NCORES + c]) for k in range(SPC)] for c in range(NCORES)], NRS, NAS


# ---------------------------------------------------------------- device
def _build(NRS, NAS):
    import concourse.bacc as bacc
    import concourse.mybir as mybir
    from concourse import tile

    f32 = mybir.dt.float32
    i16 = mybir.dt.int16
    nc = bacc.Bacc()
    NR1 = max(NRS, 1); NA1 = max(NAS, 1)
    d = {}
    d['imgs'] = nc.dram_tensor('imgs', [NA1, C, H, W], f32, kind='ExternalInput')
    d['noise'] = nc.dram_tensor('noise', [NA1, C, H, W], f32, kind='ExternalInput')
    d['scal'] = nc.dram_tensor('scal', [128, SPC, 8], f32, kind='ExternalInput')
    d['cm'] = nc.dram_tensor('cm', [SPC, 128, W], f32, kind='ExternalInput')
    d['rm'] = nc.dram_tensor('rm', [SPC, 128, 4], f32, kind='ExternalInput')
    d['pat'] = nc.dram_tensor('pat', [NR1, RPS, 128, PLANE], f32, kind='ExternalInput')
    d['idx'] = nc.dram_tensor('idx', [NR1, 128, RPS, NI // 16], i16, kind='ExternalInput')
    d['w4'] = nc.dram_tensor('w4', [NR1, RPS, 128, NI], f32, kind='ExternalInput')
    d['msk'] = nc.dram_tensor('msk', [NR1, RPS, 8, NI], f32, kind='ExternalInput')
    d['nzb'] = nc.dram_tensor('nzb', [NR1, RPS, 8, 3 * NI], f32, kind='ExternalInput')
    d['wvT'] = nc.dram_tensor('wvT', [NA1, H, H], f32, kind='ExternalInput')
    d['wh'] = nc.dram_tensor('wh', [NA1, W, W], f32, kind='ExternalInput')
    d['ident'] = nc.dram_tensor('ident', [128, 128], f32, kind='ExternalInput')
    d['smat'] = nc.dram_tensor('smat', [128, 128], f32, kind='ExternalInput')
    out_d = nc.dram_tensor('out', [SPC, C, H, W], f32, kind='ExternalOutput')

    mult = mybir.AluOpType.mult
    add = mybir.AluOpType.add
    amin = mybir.AluOpType.min
    amax = mybir.AluOpType.max
    sub = mybir.AluOpType.subtract

    with tile.TileContext(nc) as tc:
        with (
            tc.tile_pool(name='const', bufs=1) as cpool,
            tc.tile_pool(name='plane', bufs=2) as plpool,
            tc.tile_pool(name='rotio', bufs=2) as riopool,
            tc.tile_pool(name='rpost', bufs=1) as rppool,
            tc.tile_pool(name='ax', bufs=1) as apool,
            tc.tile_pool(name='post', bufs=1) as ppool,
            tc.tile_pool(name='rpsum', bufs=1, space='PSUM') as rpspool,
            tc.tile_pool(name='psum', bufs=2, space='PSUM') as pspool,
        ):
            ident = cpool.tile([128, 128], f32, tag='ident')
            nc.sync.dma_start(ident[:], d['ident'][:])
            smat = cpool.tile([128, 128], f32, tag='smat')
            nc.sync.dma_start(smat[:], d['smat'][:])
            sc_sb = cpool.tile([128, SPC, 8], f32, tag='sc')
            nc.sync.dma_start(sc_sb[:], d['scal'][:])

            # ---------------- rotated samples (gather path) ----------------
            for s in range(NRS):
                ix_s = cpool.tile([128, RPS, NI // 16], i16, tag='ix')
                nc.sync.dma_start(ix_s[:], d['idx'][s])
                for t in range(RPS):
                    P = plpool.tile([128, PLANE], f32, tag='P')
                    nc.sync.dma_start(P[:], d['pat'][s, t])
                    WT = riopool.tile([128, NI], f32, tag='WT')
                    nc.scalar.dma_start(WT[:], d['w4'][s, t])
                    G = riopool.tile([128, NI], f32, tag='G')
                    nc.gpsimd.ap_gather(
                        G[:].rearrange("p (n i) -> p n i", i=1),
                        P[:].rearrange("p (n i) -> p n i", i=1),
                        ix_s[:, t, :], channels=128, num_elems=PLANE, d=1,
                        num_idxs=NI)
                    nc.vector.tensor_tensor(G[:], G[:], WT[:], op=mult)
                    ps = rpspool.tile([8, 3 * NI], f32, tag='ps')
                    for c3 in range(3):
                        for i in range(NI // 512):
                            nc.tensor.matmul(
                                ps[:, c3 * NI + i * 512:c3 * NI + (i + 1) * 512],
                                smat[:, 8 * c3:8 * c3 + 8],
                                G[:, i * 512:(i + 1) * 512], start=True, stop=True)
                    # ---- fused post: brightness(+contrast in w)/clip ----
                    X = rppool.tile([8, 3 * NI], f32, tag='X')
                    nc.vector.tensor_scalar(
                        X[:], ps[:], sc_sb[0:8, s, 1:2], sc_sb[0:8, s, 2:3],
                        op0=add, op1=amin)
                    nc.vector.tensor_scalar(
                        X[:], X[:], sc_sb[0:8, s, 5:6], None, op0=amax)
                    gray = rppool.tile([8, NI], f32, tag='gray')
                    nc.vector.tensor_tensor(gray[:], X[:, 0:NI], X[:, NI:2 * NI], op=add)
                    nc.vector.tensor_tensor(gray[:], gray[:], X[:, 2 * NI:3 * NI], op=add)
                    nc.vector.tensor_scalar(gray[:], gray[:], sc_sb[0:8, s, 4:5],
                                            None, op0=mult)
                    for c3 in range(3):
                        nc.vector.scalar_tensor_tensor(
                            X[:, c3 * NI:(c3 + 1) * NI], X[:, c3 * NI:(c3 + 1) * NI],
                            sc_sb[0:8, s, 3:4], gray[:],
                            op0=mult, op1=add)
                    nc.vector.tensor_scalar(X[:], X[:], 1.0, -1.0, op0=amin, op1=amax)
                    # cutout: X += msk * (nzb - X)
                    NZ = rppool.tile([8, 3 * NI], f32, tag='NZ')
                    MK = rppool.tile([8, NI], f32, tag='MK')
                    nc.scalar.dma_start(NZ[:], d['nzb'][s, t])
                    nc.scalar.dma_start(MK[:], d['msk'][s, t])
                    nc.vector.tensor_tensor(NZ[:], NZ[:], X[:], op=sub)
                    for c3 in range(3):
                        nc.vector.tensor_tensor(NZ[:, c3 * NI:(c3 + 1) * NI],
                                                NZ[:, c3 * NI:(c3 + 1) * NI],
                                                MK[:], op=mult)
                    nc.vector.tensor_tensor(X[:], X[:], NZ[:], op=add)
                    bi, bj0 = (t * 8) // GRID, (t * 8) % GRID
                    for c3 in range(3):
                        nc.sync.dma_start(
                            out_d[s, c3, bi * BLK:(bi + 1) * BLK,
                                  bj0 * BLK:(bj0 + 8) * BLK].rearrange(
                                "r (g c) -> g r c", g=8),
                            X[:, c3 * NI:(c3 + 1) * NI].rearrange(
                                "g (r c) -> g r c", r=BLK))

            # ---------------- axis-aligned samples (PE path) ----------------
            for ka in range(NAS):
                s = NRS + ka
                wv_sb = apool.tile([128, 4, H], f32, tag='wv')
                wh_sb = apool.tile([128, 4, W], f32, tag='wh')
                nc.sync.dma_start(wv_sb[:], d['wvT'][ka].rearrange("(t p) i -> p t i", p=128))
                nc.sync.dma_start(wh_sb[:], d['wh'][ka].rearrange("(t p) j -> p t j", p=128))
                cm_sb = ppool.tile([128, W], f32, tag='cm')
                rm_sb = ppool.tile([128, 4], f32, tag='rm')
                nc.sync.dma_start(cm_sb[:], d['cm'][s])
                nc.sync.dma_start(rm_sb[:], d['rm'][s])
                Ot = []
                for ch in range(C):
                    img_sb = apool.tile([128, 4, W], f32, tag='img')
                    nc.sync.dma_start(img_sb[:], d['imgs'][ka, ch].rearrange("(t p) c -> p t c", p=128))
                    v_sb = apool.tile([128, 4, W], f32, tag='v')
                    for mi in range(4):
                        vps = pspool.tile([128, W], f32, tag='ps')
                        for kt in range(4):
                            nc.tensor.matmul(
                                vps[:], wv_sb[:, kt, mi * 128:(mi + 1) * 128],
                                img_sb[:, kt, :], start=(kt == 0), stop=(kt == 3))
                        nc.scalar.copy(v_sb[:, mi, :], vps[:])
                    vT_sb = apool.tile([128, 4, H], f32, tag='vt')
                    for ct in range(4):
                        tps = pspool.tile([128, H], f32, tag='ps')
                        for it in range(4):
                            nc.tensor.transpose(
                                tps[:, it * 128:(it + 1) * 128],
                                v_sb[:, it, ct * 128:(ct + 1) * 128], ident[:])
                        nc.scalar.copy(vT_sb[:, ct, :], tps[:])
                    o_sb = apool.tile([128, 4, W], f32, tag=f'o{ch}')
                    for mi in range(4):
                        ops = pspool.tile([128, W], f32, tag='ps')
                        for ct in range(4):
                            nc.tensor.matmul(
                                ops[:], vT_sb[:, ct, mi * 128:(mi + 1) * 128],
                                wh_sb[:, ct, :], start=(ct == 0), stop=(ct == 3))
                        nc.scalar.copy(o_sb[:, mi, :], ops[:])
                    Ot.append(o_sb)
                gray = ppool.tile([128, 4, W], f32, tag='gray')
                for ch in range(C):  # brightness+contrast fused + clip
                    nc.vector.tensor_scalar(
                        Ot[ch][:], Ot[ch][:], sc_sb[:, s, 0:1], sc_sb[:, s, 1:2],
                        op0=mult, op1=add)
                    nc.vector.tensor_scalar(
                        Ot[ch][:], Ot[ch][:], sc_sb[:, s, 2:3], sc_sb[:, s, 5:6],
                        op0=amin, op1=amax)
                nc.vector.tensor_tensor(gray[:], Ot[0][:], Ot[1][:], op=add)
                nc.vector.tensor_tensor(gray[:], gray[:], Ot[2][:], op=add)
                nc.vector.tensor_scalar(gray[:], gray[:], sc_sb[:, s, 4:5], None, op0=mult)
                for ch in range(C):  # saturation lerp + clip, then cutout
                    nc.vector.scalar_tensor_tensor(
                        Ot[ch][:], Ot[ch][:], sc_sb[:, s, 3:4], gray[:],
                        op0=mult, op1=add)
                    nc.vector.tensor_scalar(
                        Ot[ch][:], Ot[ch][:], 1.0, -1.0, op0=amin, op1=amax)
                    n_sb = apool.tile([128, 4, W], f32, tag='img')
                    nc.sync.dma_start(n_sb[:], d['noise'][ka, ch].rearrange("(t p) c -> p t c", p=128))
                    nc.vector.tensor_tensor(n_sb[:], n_sb[:], Ot[ch][:], op=sub)
                    for tt in range(4):
                        nc.vector.tensor_tensor(n_sb[:, tt, :], n_sb[:, tt, :],
                                                cm_sb[:], op=mult)
                        nc.vector.scalar_tensor_tensor(
                            Ot[ch][:, tt, :], n_sb[:, tt, :], rm_sb[:, tt:tt + 1],
                            Ot[ch][:, tt, :], op0=mult, op1=add)
                    nc.sync.dma_start(
                        out_d[s, ch].rearrange("(t p) c -> p t c", p=128), Ot[ch][:])
    nc.compile()
    return nc


def kernel(**inputs):
    from concourse import bass_utils
    cores, sids, NRS, NAS = _host_prep(inputs)
    key = (NRS, NAS)
    if key not in _PROGRAM_CACHE:
        _PROGRAM_CACHE[key] = _build(NRS, NAS)
    nc = _PROGRAM_CACHE[key]
    in_maps = [{k: v for k, v in c.items()} for c in cores]
    res = bass_utils.run_bass_kernel_spmd(nc, in_maps, core_ids=list(range(NCORES)))
    out = np.zeros((B, C, H, W), np.float32)
    for c in range(NCORES):
        o = res.results[c]['out']
        for k, s in enumerate(sids[c]):
            out[s] = o[k]
    return out
